# revision 40
# baseline (speedup 1.0000x reference)
"""Trainium2 Bass kernel for Bahdanau additive attention (nn_AttentionLayer).

Reference math (per batch b; t_q=128, t_k=512, n=512, h=128):
    q_proj = query @ Wq.T + bq                    # [t_q, h]
    k_proj = keys  @ Wk.T + bk                    # [t_k, h]
    scores[i,j] = Wo[0] . tanh(q_proj[i] + k_proj[j]) (+ bo, softmax-invariant)
    attn = softmax(scores, axis=-1)
    context = attn @ values
    returns (context, attn)

Sharding: data-parallel over batch b — one batch element per NeuronCore (8 cores).

Device strategy (per core):
  * kpT[h=128, j=512] = Wk @ keys.T with hidden dim on partitions (fp32, exact).
  * qpb[h=128, i=128] = Wq @ query.T + (bq+bk) — per-query bias columns (fp32).
  * Scores loop in groups of 8 queries:
      - per query: sum_i[h, j] = kpT + qpb[:, i] on DVE/GPSIMD (tensor_scalar,
        per-partition scalar operand; DVE runs at 2x fp32 mode)
      - one big ScalarE op per group: hid = tanh(sum_group) -> bf16
        ([128, 4096]: the 128/1.2GHz-cycle ACT overhead amortizes 8x)
      - one bf16 TensorE matmul per query with a zero-padded stationary weight
        (lhsT = wo_shift[:, i%32, :]; Wo in column i%32) accumulating scores
        into rows of one [128, 512] PSUM tile => natural [i, j] layout.
        bf16 moving operand streams 1 cycle/row (fp32 would take 4).
  * Softmax: Exp with accum_out (free-dim row-sum) -> reciprocal -> scale.
  * context = (exp @ values) * recip via 4 PE transposes of exp + 4 fp32
    matmuls; the normalization rides the PSUM->SBUF copy for free.
"""

from contextlib import ExitStack

import ml_dtypes
import numpy as np

import concourse.bass as bass
import concourse.tile as tile
from concourse import bacc, masks, mybir
from concourse.bass_utils import run_bass_kernel_spmd

F32 = mybir.dt.float32
F32R = mybir.dt.float32r
BF16 = mybir.dt.bfloat16
AF = mybir.ActivationFunctionType

B = 8          # batch (== number of cores)
TQ = 128       # query positions
TK = 512       # key positions
NQ = 512       # query feature dim
NK = 512       # key feature dim
NV = 512       # value feature dim
H = 128        # hidden dim
STRIP = 32     # query strip width (PE column-group granularity)
# The first SINGLES queries run fused add+tanh on ScalarE (per-partition
# bias) with no DVE dependency — they start immediately after the
# projections while the DVE add pipeline builds a lead, and they shift a
# little work from DVE (the aggregate bottleneck) to ScalarE.
SINGLES = 7
# Tanh group sizes (queries per ScalarE op) for the remaining queries.
# Small groups last so the PE's final matmul burst (which can only start
# after the group's tanh) stays short; 16-wide groups amortize the
# ~312-cycle ACT per-op overhead 16x. Boundaries tile into 32-query strips.
GROUPS = [9, 16] + [16] * 5 + [8, 4, 4]

_CACHE: dict = {}


def _build_nc() -> bass.Bass:
    nc = bacc.Bacc("TRN2", target_bir_lowering=False, debug=False)

    # queryT/keysT are host-side layout marshalling of the per-core shard
    # (feature dim leading) so the contraction dim lands on SBUF partitions
    # without on-device transposes.
    qt_d = nc.dram_tensor("queryT", [NQ, TQ], F32, kind="ExternalInput")
    kt_d = nc.dram_tensor("keysT", [NK, TK], F32, kind="ExternalInput")
    v_d = nc.dram_tensor("values", [TK, NV], F32R, kind="ExternalInput")
    wqt_d = nc.dram_tensor("WqT", [NQ, H], F32, kind="ExternalInput")
    wkt_d = nc.dram_tensor("WkT", [NK, H], F32, kind="ExternalInput")
    bqk_d = nc.dram_tensor("bqk", [H, 1], F32, kind="ExternalInput")
    wosh_d = nc.dram_tensor("wo_shift", [H, STRIP, STRIP], BF16, kind="ExternalInput")
    ctx_d = nc.dram_tensor("context", [TQ, NV], F32, kind="ExternalOutput")
    attn_d = nc.dram_tensor("attn", [TQ, TK], F32, kind="ExternalOutput")

    KC = NK // 128  # 4 contraction chunks over the feature dim
    JC = TK // 128  # 4 chunks over key positions

    with tile.TileContext(nc) as tc:
        with ExitStack() as ctx:
            consts = ctx.enter_context(tc.tile_pool(name="consts", bufs=1))
            ins = ctx.enter_context(tc.tile_pool(name="ins", bufs=1))
            tp_ps = ctx.enter_context(
                tc.tile_pool(name="tp_ps", bufs=2, space=bass.MemorySpace.PSUM)
            )
            proj_ps = ctx.enter_context(
                tc.tile_pool(name="proj_ps", bufs=1, space=bass.MemorySpace.PSUM)
            )
            score_ps = ctx.enter_context(
                tc.tile_pool(name="score_ps", bufs=1, space=bass.MemorySpace.PSUM)
            )
            ctx_ps = ctx.enter_context(
                tc.tile_pool(name="ctx_ps", bufs=1, space=bass.MemorySpace.PSUM)
            )
            warm_ps = ctx.enter_context(
                tc.tile_pool(name="warm_ps", bufs=1, space=bass.MemorySpace.PSUM)
            )
            sum_pool = ctx.enter_context(tc.tile_pool(name="sumg", bufs=3))
            hid_pool = ctx.enter_context(tc.tile_pool(name="hidg", bufs=2))
            sm_pool = ctx.enter_context(tc.tile_pool(name="sm", bufs=1))
            att_pool = ctx.enter_context(tc.tile_pool(name="attT", bufs=2))

            # ---- inputs (order matters: keys/query feed the critical path) ----
            # Big loads split across queues; weight loads dispatched from the
            # (otherwise idle) ScalarE HWDGE so dispatches run in parallel
            # with the sync-engine ones (~650ns dispatch each, serial per
            # engine).
            with nc.named_scope("load"):
                kT = ins.tile([128, KC, TK], F32, tag="kT")
                kt_src = kt_d.ap().rearrange("(c p) j -> p c j", p=128)
                for c in range(KC):
                    eng = nc.sync if c % 2 == 0 else nc.scalar
                    eng.dma_start(kT[:, c : c + 1, :], kt_src[:, c : c + 1, :])
                qT = ins.tile([128, KC, TQ], F32, tag="qT")
                nc.sync.dma_start(
                    qT[:], qt_d.ap().rearrange("(c p) i -> p c i", p=128)
                )
                wkt = consts.tile([128, KC, H], F32, tag="wkt")
                nc.scalar.dma_start(
                    wkt[:], wkt_d.ap().rearrange("(c p) h -> p c h", p=128)
                )
                wqt = consts.tile([128, KC, H], F32, tag="wqt")
                nc.scalar.dma_start(
                    wqt[:], wqt_d.ap().rearrange("(c p) h -> p c h", p=128)
                )
                bqk = consts.tile([H, 1], F32, tag="bqk")
                nc.scalar.dma_start(bqk[:], bqk_d.ap())
                wosh = consts.tile([H, STRIP, STRIP], BF16, tag="wosh")
                nc.scalar.dma_start(wosh[:], wosh_d.ap())
                v_sb = ins.tile([128, JC, NV], F32R, tag="v_sb")
                nc.sync.dma_start(
                    v_sb[:], v_d.ap().rearrange("(r p) n -> p r n", p=128)
                )
                ident = consts.tile([128, 128], F32, tag="ident")
                masks.make_identity(nc, ident[:])
                # PE warm-up: ~2-3us of throwaway matmuls while the input DMAs
                # land, so HAM un-throttles the clock (1.2 -> 2.4 GHz) before
                # the projection matmuls issue (kept short: these occupy the
                # PE FIFO ahead of the projections).
                wps = warm_ps.tile([128, 128], F32, tag="warm")
                for _ in range(6):
                    nc.tensor.matmul(wps[:], ident[:], ident[:], start=True, stop=True)

            # ---- projections (fp32, exact: these feed the tanh input) ----
            with nc.named_scope("proj"):
                kpT_ps = proj_ps.tile([H, TK], F32, tag="kpT")
                for c in range(KC):
                    nc.tensor.matmul(
                        kpT_ps[:],
                        wkt[:, c, :],
                        kT[:, c, :],
                        start=(c == 0),
                        stop=(c == KC - 1),
                    )
                kpT = consts.tile([H, TK], F32, tag="kpT_sb")
                nc.scalar.copy(kpT[:], kpT_ps[:])
                qp_ps = proj_ps.tile([H, TQ], F32, tag="qp")
                for c in range(KC):
                    nc.tensor.matmul(
                        qp_ps[:],
                        wqt[:, c, :],
                        qT[:, c, :],
                        start=(c == 0),
                        stop=(c == KC - 1),
                    )
                qpb = consts.tile([H, TQ], F32, tag="qpb")
                nc.scalar.activation(qpb[:], qp_ps[:], AF.Identity, bias=bqk[:, 0:1])

            # ---- scores ----
            # ST[i, j] accumulates in natural layout via zero-padded bf16
            # stationary weights; strips must run in order (PSUM has_written
            # is cleared bank-wide by each accumulation-group start).
            with nc.named_scope("scores"):
                st = score_ps.tile([TQ, TK], F32, tag="st")

                def score_mm(i, hid_ap):
                    s, qq = i // STRIP, i % STRIP
                    nc.tensor.matmul(
                        st[s * STRIP : (s + 1) * STRIP, :],
                        wosh[:, qq, :],
                        hid_ap,
                        start=(qq == 0),
                        stop=(qq == STRIP - 1),
                        tile_position=(0, s * STRIP),
                    )

                # fused add+tanh singles (read kpT straight from PSUM)
                for i in range(SINGLES):
                    hid1 = hid_pool.tile([H, TK], BF16, tag="hid1")
                    nc.scalar.activation(
                        hid1[:], kpT_ps[:], AF.Tanh, bias=qpb[:, i : i + 1]
                    )
                    score_mm(i, hid1[:])

                assert SINGLES + sum(GROUPS) == TQ
                i0 = SINGLES
                for g_sz in GROUPS:
                    sum_t = sum_pool.tile([H, g_sz * TK], F32, tag="sumg")
                    for q in range(g_sz):
                        nc.vector.tensor_scalar_add(
                            sum_t[:, q * TK : (q + 1) * TK],
                            kpT[:],
                            qpb[:, i0 + q : i0 + q + 1],
                        )
                    hid = hid_pool.tile([H, g_sz * TK], BF16, tag="hidg")
                    nc.scalar.activation(hid[:], sum_t[:], AF.Tanh)
                    for q in range(g_sz):
                        score_mm(i0 + q, hid[:, q * TK : (q + 1) * TK])
                    if g_sz >= 16:
                        # PE keep-warm: enough dummy work per group that the
                        # idle stretch stays under HAM's 3.4us MID window.
                        wps = warm_ps.tile([128, TK], F32, tag="warm")
                        for _ in range(5):
                            nc.tensor.matmul(
                                wps[:, :TK],
                                hid[:, 0:128],
                                hid[:, 0:TK],
                                start=True,
                                stop=True,
                            )
                    i0 += g_sz

            # ---- softmax (no max-subtraction needed: |scores| <= ~12) ----
            with nc.named_scope("softmax"):
                exp_sb = sm_pool.tile([TQ, TK], F32, tag="exp")
                denom = sm_pool.tile([TQ, 1], F32, tag="denom")
                nc.scalar.activation(exp_sb[:], st[:], AF.Exp, accum_out=denom[:])
                recip = sm_pool.tile([TQ, 1], F32, tag="recip")
                nc.vector.reciprocal(recip[:], denom[:])
                attn_sb = sm_pool.tile([TQ, TK], F32, tag="attn")
                nc.vector.tensor_scalar_mul(attn_sb[:], exp_sb[:], recip[:, 0:1])
                nc.sync.dma_start(attn_d.ap(), attn_sb[:])

            # ---- context = (exp @ values) * recip ----
            with nc.named_scope("context"):
                expT = []
                for c in range(JC):
                    pst = tp_ps.tile([128, 128], F32, tag="tpp")
                    nc.tensor.transpose(
                        pst[:], exp_sb[:, c * 128 : (c + 1) * 128], ident[:]
                    )
                    t = att_pool.tile([128, TQ], F32R, tag="expT")
                    nc.scalar.copy(t[:], pst[:])
                    expT.append(t)
                # float32r: single-pass matmul (fp32 takes 4 cycles/row as a
                # LOW_HIGH pair). attn is always positive and values have
                # random signs, so the reduced-precision product error stays
                # ~1e-4 RMS on context — same class as the bf16 scores path.
                cps = ctx_ps.tile([TQ, NV], F32, tag="ctx")
                for c in range(JC):
                    nc.tensor.matmul(
                        cps[:],
                        expT[c][:],
                        v_sb[:, c, :],
                        start=(c == 0),
                        stop=(c == JC - 1),
                    )
                ctx_sb = sm_pool.tile([TQ, NV], F32, tag="ctx_sb")
                nc.vector.tensor_scalar_mul(ctx_sb[:], cps[:], recip[:, 0:1])
                nc.sync.dma_start(ctx_d.ap(), ctx_sb[:])

    nc.finalize()
    return nc


def _get_nc() -> bass.Bass:
    if "nc" not in _CACHE:
        _CACHE["nc"] = _build_nc()
    return _CACHE["nc"]


def _prep_in_maps(query, keys, values, Wq, bq, Wk, bk, Wo, bo):
    WqT = np.ascontiguousarray(np.asarray(Wq, np.float32).T)
    WkT = np.ascontiguousarray(np.asarray(Wk, np.float32).T)
    bqk = (np.asarray(bq, np.float32) + np.asarray(bk, np.float32)).reshape(H, 1)
    wo_shift = np.zeros((H, STRIP, STRIP), np.float32)
    idx = np.arange(STRIP)
    wo_shift[:, idx, idx] = np.asarray(Wo, np.float32)[0][:, None]
    wo_shift = np.ascontiguousarray(wo_shift.astype(ml_dtypes.bfloat16))
    query = np.asarray(query, np.float32)
    keys = np.asarray(keys, np.float32)
    values = np.asarray(values, np.float32)
    in_maps = []
    for b in range(B):
        in_maps.append(
            {
                "queryT": np.ascontiguousarray(query[b].T),
                "keysT": np.ascontiguousarray(keys[b].T),
                "values": np.ascontiguousarray(values[b]),
                "WqT": WqT,
                "WkT": WkT,
                "bqk": bqk,
                "wo_shift": wo_shift,
            }
        )
    return in_maps


def _run(inputs: dict, trace: bool = False):
    nc = _get_nc()
    in_maps = _prep_in_maps(**inputs)
    try:
        res = run_bass_kernel_spmd(nc, in_maps, core_ids=list(range(B)), trace=trace)
    except Exception:
        if not trace:
            raise
        import traceback

        traceback.print_exc()
        print("trace run failed; falling back to untraced run")
        res = run_bass_kernel_spmd(nc, in_maps, core_ids=list(range(B)), trace=False)
    context = np.stack([res.results[b]["context"] for b in range(B)])
    attn = np.stack([res.results[b]["attn"] for b in range(B)])
    return (context, attn), res


def kernel(**inputs):
    (context, attn), _ = _run(inputs, trace=False)
    return context, attn


# revision 42
# speedup vs baseline: 1.0483x; 1.0483x over previous
"""Trainium2 Bass kernel for Bahdanau additive attention (nn_AttentionLayer).

Reference math (per batch b; t_q=128, t_k=512, n=512, h=128):
    q_proj = query @ Wq.T + bq                    # [t_q, h]
    k_proj = keys  @ Wk.T + bk                    # [t_k, h]
    scores[i,j] = Wo[0] . tanh(q_proj[i] + k_proj[j]) (+ bo, softmax-invariant)
    attn = softmax(scores, axis=-1)
    context = attn @ values
    returns (context, attn)

Sharding: data-parallel over batch b — one batch element per NeuronCore (8 cores).

Device strategy (per core):
  * kpT[h=128, j=512] = Wk @ keys.T with hidden dim on partitions (fp32, exact).
  * qpb[h=128, i=128] = Wq @ query.T + (bq+bk) — per-query bias columns (fp32).
  * Scores loop in groups of 8 queries:
      - per query: sum_i[h, j] = kpT + qpb[:, i] on DVE/GPSIMD (tensor_scalar,
        per-partition scalar operand; DVE runs at 2x fp32 mode)
      - one big ScalarE op per group: hid = tanh(sum_group) -> bf16
        ([128, 4096]: the 128/1.2GHz-cycle ACT overhead amortizes 8x)
      - one bf16 TensorE matmul per query with a zero-padded stationary weight
        (lhsT = wo_shift[:, i%32, :]; Wo in column i%32) accumulating scores
        into rows of one [128, 512] PSUM tile => natural [i, j] layout.
        bf16 moving operand streams 1 cycle/row (fp32 would take 4).
  * Softmax: Exp with accum_out (free-dim row-sum) -> reciprocal -> scale.
  * context = (exp @ values) * recip via 4 PE transposes of exp + 4 fp32
    matmuls; the normalization rides the PSUM->SBUF copy for free.
"""

from contextlib import ExitStack

import ml_dtypes
import numpy as np

import concourse.bass as bass
import concourse.tile as tile
from concourse import bacc, masks, mybir
from concourse.bass_utils import run_bass_kernel_spmd

F32 = mybir.dt.float32
F32R = mybir.dt.float32r
BF16 = mybir.dt.bfloat16
AF = mybir.ActivationFunctionType

B = 8          # batch (== number of cores)
TQ = 128       # query positions
TK = 512       # key positions
NQ = 512       # query feature dim
NK = 512       # key feature dim
NV = 512       # value feature dim
H = 128        # hidden dim
STRIP = 32     # query strip width (PE column-group granularity)
# The first SINGLES queries run fused add+tanh on ScalarE (per-partition
# bias) with no DVE dependency — they start immediately after the
# projections while the DVE add pipeline builds a lead, and they shift a
# little work from DVE (the aggregate bottleneck) to ScalarE.
SINGLES = 7
# Tanh group sizes (queries per ScalarE op) for the remaining queries.
# Small groups last so the PE's final matmul burst (which can only start
# after the group's tanh) stays short; 16-wide groups amortize the
# ~312-cycle ACT per-op overhead 16x. Boundaries tile into 32-query strips.
GROUPS = [9, 16] + [16] * 5 + [8, 4, 4]

_CACHE: dict = {}


def _build_nc() -> bass.Bass:
    nc = bacc.Bacc("TRN2", target_bir_lowering=False, debug=False)

    # queryT/keysT are host-side layout marshalling of the per-core shard
    # (feature dim leading) so the contraction dim lands on SBUF partitions
    # without on-device transposes.
    qt_d = nc.dram_tensor("queryT", [NQ, TQ], F32, kind="ExternalInput")
    kt_d = nc.dram_tensor("keysT", [NK, TK], F32, kind="ExternalInput")
    v_d = nc.dram_tensor("values", [TK, NV], F32R, kind="ExternalInput")
    wqt_d = nc.dram_tensor("WqT", [NQ, H], F32, kind="ExternalInput")
    wkt_d = nc.dram_tensor("WkT", [NK, H], F32, kind="ExternalInput")
    bqk_d = nc.dram_tensor("bqk", [H, 1], F32, kind="ExternalInput")
    wosh_d = nc.dram_tensor("wo_shift", [H, STRIP, STRIP], BF16, kind="ExternalInput")
    ctx_d = nc.dram_tensor("context", [TQ, NV], F32, kind="ExternalOutput")
    attn_d = nc.dram_tensor("attn", [TQ, TK], F32, kind="ExternalOutput")

    KC = NK // 128  # 4 contraction chunks over the feature dim
    JC = TK // 128  # 4 chunks over key positions

    with tile.TileContext(nc) as tc:
        with ExitStack() as ctx:
            consts = ctx.enter_context(tc.tile_pool(name="consts", bufs=1))
            ins = ctx.enter_context(tc.tile_pool(name="ins", bufs=1))
            tp_ps = ctx.enter_context(
                tc.tile_pool(name="tp_ps", bufs=2, space=bass.MemorySpace.PSUM)
            )
            proj_ps = ctx.enter_context(
                tc.tile_pool(name="proj_ps", bufs=1, space=bass.MemorySpace.PSUM)
            )
            score_ps = ctx.enter_context(
                tc.tile_pool(name="score_ps", bufs=1, space=bass.MemorySpace.PSUM)
            )
            ctx_ps = ctx.enter_context(
                tc.tile_pool(name="ctx_ps", bufs=1, space=bass.MemorySpace.PSUM)
            )
            warm_ps = ctx.enter_context(
                tc.tile_pool(name="warm_ps", bufs=1, space=bass.MemorySpace.PSUM)
            )
            sum_pool = ctx.enter_context(tc.tile_pool(name="sumg", bufs=3))
            hid_pool = ctx.enter_context(tc.tile_pool(name="hidg", bufs=2))
            sm_pool = ctx.enter_context(tc.tile_pool(name="sm", bufs=1))
            att_pool = ctx.enter_context(tc.tile_pool(name="attT", bufs=2))

            # ---- inputs (order matters: keys/query feed the critical path) ----
            # Big loads split across queues; weight loads dispatched from the
            # (otherwise idle) ScalarE HWDGE so dispatches run in parallel
            # with the sync-engine ones (~650ns dispatch each, serial per
            # engine).
            with nc.named_scope("load"):
                kT = ins.tile([128, KC, TK], F32, tag="kT")
                kt_src = kt_d.ap().rearrange("(c p) j -> p c j", p=128)
                for c in range(KC):
                    nc.sync.dma_start(kT[:, c : c + 1, :], kt_src[:, c : c + 1, :])
                qT = ins.tile([128, KC, TQ], F32, tag="qT")
                nc.sync.dma_start(
                    qT[:], qt_d.ap().rearrange("(c p) i -> p c i", p=128)
                )
                wkt = consts.tile([128, KC, H], F32, tag="wkt")
                nc.scalar.dma_start(
                    wkt[:], wkt_d.ap().rearrange("(c p) h -> p c h", p=128)
                )
                wqt = consts.tile([128, KC, H], F32, tag="wqt")
                nc.scalar.dma_start(
                    wqt[:], wqt_d.ap().rearrange("(c p) h -> p c h", p=128)
                )
                bqk = consts.tile([H, 1], F32, tag="bqk")
                nc.scalar.dma_start(bqk[:], bqk_d.ap())
                wosh = consts.tile([H, STRIP, STRIP], BF16, tag="wosh")
                nc.scalar.dma_start(wosh[:], wosh_d.ap())
                v_sb = ins.tile([128, JC, NV], F32R, tag="v_sb")
                nc.sync.dma_start(
                    v_sb[:], v_d.ap().rearrange("(r p) n -> p r n", p=128)
                )
                ident = consts.tile([128, 128], F32, tag="ident")
                masks.make_identity(nc, ident[:])
                # PE warm-up: ~2-3us of throwaway matmuls while the input DMAs
                # land, so HAM un-throttles the clock (1.2 -> 2.4 GHz) before
                # the projection matmuls issue (kept short: these occupy the
                # PE FIFO ahead of the projections).
                wps = warm_ps.tile([128, 128], F32, tag="warm")
                for _ in range(5):
                    nc.tensor.matmul(wps[:], ident[:], ident[:], start=True, stop=True)

            # ---- projections (fp32, exact: these feed the tanh input) ----
            with nc.named_scope("proj"):
                kpT_ps = proj_ps.tile([H, TK], F32, tag="kpT")
                for c in range(KC):
                    nc.tensor.matmul(
                        kpT_ps[:],
                        wkt[:, c, :],
                        kT[:, c, :],
                        start=(c == 0),
                        stop=(c == KC - 1),
                    )
                kpT = consts.tile([H, TK], F32, tag="kpT_sb")
                nc.scalar.copy(kpT[:], kpT_ps[:])
                qp_ps = proj_ps.tile([H, TQ], F32, tag="qp")
                for c in range(KC):
                    nc.tensor.matmul(
                        qp_ps[:],
                        wqt[:, c, :],
                        qT[:, c, :],
                        start=(c == 0),
                        stop=(c == KC - 1),
                    )
                qpb = consts.tile([H, TQ], F32, tag="qpb")
                nc.scalar.activation(qpb[:], qp_ps[:], AF.Identity, bias=bqk[:, 0:1])

            # ---- scores ----
            # ST[i, j] accumulates in natural layout via zero-padded bf16
            # stationary weights; strips must run in order (PSUM has_written
            # is cleared bank-wide by each accumulation-group start).
            with nc.named_scope("scores"):
                st = score_ps.tile([TQ, TK], F32, tag="st")

                def score_mm(i, hid_ap):
                    s, qq = i // STRIP, i % STRIP
                    nc.tensor.matmul(
                        st[s * STRIP : (s + 1) * STRIP, :],
                        wosh[:, qq, :],
                        hid_ap,
                        start=(qq == 0),
                        stop=(qq == STRIP - 1),
                        tile_position=(0, s * STRIP),
                    )

                # fused add+tanh singles (read kpT straight from PSUM)
                for i in range(SINGLES):
                    hid1 = hid_pool.tile([H, TK], BF16, tag="hid1")
                    nc.scalar.activation(
                        hid1[:], kpT_ps[:], AF.Tanh, bias=qpb[:, i : i + 1]
                    )
                    score_mm(i, hid1[:])

                assert SINGLES + sum(GROUPS) == TQ
                i0 = SINGLES
                for g_sz in GROUPS:
                    sum_t = sum_pool.tile([H, g_sz * TK], F32, tag="sumg")
                    for q in range(g_sz):
                        nc.vector.tensor_scalar_add(
                            sum_t[:, q * TK : (q + 1) * TK],
                            kpT[:],
                            qpb[:, i0 + q : i0 + q + 1],
                        )
                    hid = hid_pool.tile([H, g_sz * TK], BF16, tag="hidg")
                    nc.scalar.activation(hid[:], sum_t[:], AF.Tanh)
                    for q in range(g_sz):
                        score_mm(i0 + q, hid[:, q * TK : (q + 1) * TK])
                    if g_sz >= 16:
                        # PE keep-warm: enough dummy work per group that the
                        # idle stretch stays under HAM's 3.4us MID window.
                        wps = warm_ps.tile([128, TK], F32, tag="warm")
                        for _ in range(5):
                            nc.tensor.matmul(
                                wps[:, :TK],
                                hid[:, 0:128],
                                hid[:, 0:TK],
                                start=True,
                                stop=True,
                            )
                    i0 += g_sz

            # ---- softmax (no max-subtraction needed: |scores| <= ~12) ----
            with nc.named_scope("softmax"):
                exp_sb = sm_pool.tile([TQ, TK], F32, tag="exp")
                denom = sm_pool.tile([TQ, 1], F32, tag="denom")
                nc.scalar.activation(exp_sb[:], st[:], AF.Exp, accum_out=denom[:])
                recip = sm_pool.tile([TQ, 1], F32, tag="recip")
                nc.vector.reciprocal(recip[:], denom[:])
                attn_sb = sm_pool.tile([TQ, TK], F32, tag="attn")
                nc.vector.tensor_scalar_mul(attn_sb[:], exp_sb[:], recip[:, 0:1])
                nc.sync.dma_start(attn_d.ap(), attn_sb[:])

            # ---- context = (exp @ values) * recip ----
            with nc.named_scope("context"):
                expT = []
                for c in range(JC):
                    pst = tp_ps.tile([128, 128], F32, tag="tpp")
                    nc.tensor.transpose(
                        pst[:], exp_sb[:, c * 128 : (c + 1) * 128], ident[:]
                    )
                    t = att_pool.tile([128, TQ], F32R, tag="expT")
                    nc.scalar.copy(t[:], pst[:])
                    expT.append(t)
                # float32r: single-pass matmul (fp32 takes 4 cycles/row as a
                # LOW_HIGH pair). attn is always positive and values have
                # random signs, so the reduced-precision product error stays
                # ~1e-4 RMS on context — same class as the bf16 scores path.
                cps = ctx_ps.tile([TQ, NV], F32, tag="ctx")
                for c in range(JC):
                    nc.tensor.matmul(
                        cps[:],
                        expT[c][:],
                        v_sb[:, c, :],
                        start=(c == 0),
                        stop=(c == JC - 1),
                    )
                ctx_sb = sm_pool.tile([TQ, NV], F32, tag="ctx_sb")
                nc.vector.tensor_scalar_mul(ctx_sb[:], cps[:], recip[:, 0:1])
                nc.sync.dma_start(ctx_d.ap(), ctx_sb[:])

    nc.finalize()
    return nc


def _get_nc() -> bass.Bass:
    if "nc" not in _CACHE:
        _CACHE["nc"] = _build_nc()
    return _CACHE["nc"]


def _prep_in_maps(query, keys, values, Wq, bq, Wk, bk, Wo, bo):
    WqT = np.ascontiguousarray(np.asarray(Wq, np.float32).T)
    WkT = np.ascontiguousarray(np.asarray(Wk, np.float32).T)
    bqk = (np.asarray(bq, np.float32) + np.asarray(bk, np.float32)).reshape(H, 1)
    wo_shift = np.zeros((H, STRIP, STRIP), np.float32)
    idx = np.arange(STRIP)
    wo_shift[:, idx, idx] = np.asarray(Wo, np.float32)[0][:, None]
    wo_shift = np.ascontiguousarray(wo_shift.astype(ml_dtypes.bfloat16))
    query = np.asarray(query, np.float32)
    keys = np.asarray(keys, np.float32)
    values = np.asarray(values, np.float32)
    in_maps = []
    for b in range(B):
        in_maps.append(
            {
                "queryT": np.ascontiguousarray(query[b].T),
                "keysT": np.ascontiguousarray(keys[b].T),
                "values": np.ascontiguousarray(values[b]),
                "WqT": WqT,
                "WkT": WkT,
                "bqk": bqk,
                "wo_shift": wo_shift,
            }
        )
    return in_maps


def _run(inputs: dict, trace: bool = False):
    nc = _get_nc()
    in_maps = _prep_in_maps(**inputs)
    try:
        res = run_bass_kernel_spmd(nc, in_maps, core_ids=list(range(B)), trace=trace)
    except Exception:
        if not trace:
            raise
        import traceback

        traceback.print_exc()
        print("trace run failed; falling back to untraced run")
        res = run_bass_kernel_spmd(nc, in_maps, core_ids=list(range(B)), trace=False)
    context = np.stack([res.results[b]["context"] for b in range(B)])
    attn = np.stack([res.results[b]["attn"] for b in range(B)])
    return (context, attn), res


def kernel(**inputs):
    (context, attn), _ = _run(inputs, trace=False)
    return context, attn
